# revision 20
# baseline (speedup 1.0000x reference)
"""KAN (Jacobi/shared) kernel for Trainium2, 8 NeuronCores.

Math: y[b,o,s] = sum_{i,d} P_d(tanh(x[b,i,s])) * C[i,o,d],  P_d = Jacobi(a=1,b=1)
Monomial reformulation (host-side basis change, exact):
  => y[b,o,s] = bias[o] + sum_{k=1..4} sum_i t^k[b,i,s] * W[i,o,k]

Device pipeline (per core, fp16 I/O to halve HBM traffic; tolerance 2e-2):
  DMA x tile [128, F] (2 batches x 64 dims on partitions, F points)
  -> tanh (Act) -> t^2, t^3 (DVE muls), t^4 (Act Square or DVE)
  -> matmuls into 2-bank PSUM units:
     unpacked tiles: 4 accumulating K=64 matmuls per 512-col chunk per batch
     packed tiles:   repack [t1;t3],[t2;t4] per batch via 6 half-copies
                     (DVE 4x f16), then 2 accumulating K=128 matmuls
                     -> halves TensorE time for those tiles
  -> epilogue +bias, f32 PSUM -> f16 SBUF, spread across Act and Pool engines
  -> DMA out.

Sharding: split the 65536-point axis into 8 chunks of 8192 (data parallel),
replicate the tiny weights. Full fp32 inputs in, full fp32 output out
(fp16 conversion on host; device sees fp16).
"""

import sys

import numpy as np

if "/opt/trn_rl_repo" not in sys.path:
    sys.path.insert(0, "/opt/trn_rl_repo")

B = 4
I = 64
S = 65536
O = 128
NCORES = 8
SC = S // NCORES   # 8192 points per core
NP = (B * I) // 128  # 2 partition blocks (2 batches each)
OW = 1024            # out-DMA tile width

# coeff of t^k (rows) in Jacobi P^(1,1)_d (cols), d=0..4
_MONO = np.array(
    [
        [1.0, 0.0, -0.75, 0.0, 0.625],
        [0.0, 2.0, 0.0, -3.0, 0.0],
        [0.0, 0.0, 3.75, 0.0, -8.75],
        [0.0, 0.0, 0.0, 7.0, 0.0],
        [0.0, 0.0, 0.0, 0.0, 13.125],
    ],
    dtype=np.float64,
)

# --- tunables -------------------------------------------------------------
# F: pair-tile width. n_packed: how many pair-tiles (of NP*SC/F) use the
# K=128 packed path. t4_act_n: how many compute t^4 on Act (Square).
# epi = (u_act, u_pool, u_dve): epilogue engine unit counts out of
# 2*B*SC/QW = 32; pool-heavy early, act/dve late.
CONFIG = dict(
    F=2048,
    qw=512,
    n_packed=3,
    packed_mask=(0, 0, 0, 1, 0, 1, 0, 1),
    t4_eng=('pool', 'pool', 'pool', 'act', 'pool', 'dve', 'pool', 'dve'),
    pool_copies=(0, 0, 0, 2, 0, 2, 0, 2),
    epi_list=('act', 'dve', 'act', 'act', 'act', 'dve', 'dve', 'dve', 'act', 'dve', 'dve', 'act', 'dve', 'dve', 'act', 'act', 'act', 'dve', 'act', 'dve', 'act', 'dve', 'act', 'dve', 'dve', 'act', 'act', 'act', 'dve', 'act', 'act', 'act', 'act', 'dve', 'act', 'act', 'act', 'act', 'dve', 'act', 'act', 'act', 'act', 'dve', 'act', 'act', 'act', 'act', 'act', 'act', 'act', 'dve', 'act', 'dve', 'act', 'dve', 'act', 'dve', 'act', 'dve', 'act', 'dve', 'dve', 'act'),
    epi_spec=None,
    split0=True,
    n_warm=8,
    bufs={'xin': 4, 't1': 4, 't2': 5, 't3': 3, 't4': 3, 'a1': 2, 'b1': 3, 'out': 6, 'psum': 7},
)
# --------------------------------------------------------------------------


def _spread(n, total):
    """n of `total` slots, spread evenly, avoiding the first and last."""
    picks = set()
    if n >= total:
        return [1] * total
    inner = list(range(1, total))
    step = len(inner) / n if n else 1
    i = 0.0
    while len(picks) < n:
        picks.add(inner[min(int(i), len(inner) - 1)])
        i += step
    return [1 if k in picks else 0 for k in range(total)]


def _epi_order(spec, total):
    """Engine per epilogue unit from segmented spec:
    ((pattern, frac), (pattern, frac), ...) with fracs summing to 1."""
    units = []
    for pat, frac in spec:
        n = round(frac * total)
        for k in range(n):
            units.append(pat[k % len(pat)])
    while len(units) < total:
        units.append("act")
    return units[:total]


def _build_nc(cfg=None):
    import concourse.bacc as bacc
    import concourse.tile as tile
    from concourse import mybir

    cfg = dict(CONFIG if cfg is None else cfg)
    F = cfg["F"]
    NJ = SC // F
    NT = NP * NJ
    bufs = dict(cfg["bufs"])
    split0 = cfg["split0"]
    packed = cfg.get("packed_mask") or _spread(cfg["n_packed"], NT)
    t4_eng = cfg.get("t4_eng") or tuple(
        "act" if b else "dve"
        for b in (cfg.get("t4_mask") or _spread(cfg.get("t4_act_n", 0), NT))
    )
    pool_copies = cfg.get("pool_copies") or (0,) * NT
    QW = cfg.get("qw", 1024)
    n_units = NT * 2 * (F // QW)
    epi = cfg.get("epi_list") or _epi_order(cfg["epi_spec"], n_units)
    f32 = mybir.dt.float32
    f16 = mybir.dt.float16
    AF = mybir.ActivationFunctionType

    nc = bacc.Bacc("TRN2", target_bir_lowering=False, debug=False)

    x_dram = nc.dram_tensor("x", [B * I, SC], f16, kind="ExternalInput")
    wu_dram = nc.dram_tensor("wu", [128, 4, O], f16, kind="ExternalInput")
    wp_dram = nc.dram_tensor("wp", [128, 2, O], f16, kind="ExternalInput")
    b_dram = nc.dram_tensor("bias", [O, 1], f32, kind="ExternalInput")
    y_dram = nc.dram_tensor("y", [B, O, SC], f16, kind="ExternalOutput")

    with tile.TileContext(nc) as tc:
        with (
            tc.tile_pool(name="consts", bufs=1) as consts,
            tc.tile_pool(name="xin", bufs=bufs["xin"]) as xin_pool,
            tc.tile_pool(name="t1", bufs=bufs["t1"]) as t1_pool,
            tc.tile_pool(name="t2", bufs=bufs["t2"]) as t2_pool,
            tc.tile_pool(name="t3", bufs=bufs["t3"]) as t3_pool,
            tc.tile_pool(name="t4", bufs=bufs["t4"]) as t4_pool,
            tc.tile_pool(name="a1", bufs=bufs["a1"]) as a1_pool,
            tc.tile_pool(name="b1", bufs=bufs["b1"]) as b1_pool,
            tc.tile_pool(name="out", bufs=bufs["out"]) as out_pool,
            tc.tile_pool(name="psum", bufs=bufs["psum"], space="PSUM") as psum_pool,
        ):
            xv = x_dram.ap()  # [256, SC]
            # first x chunk before consts: tanh is the pipeline head
            xin0 = xin_pool.tile([128, F // 2], f16, name="xin")
            nc.sync.dma_start(out=xin0[:, :], in_=xv[0:128, 0 : F // 2])

            n_warm = cfg.get("n_warm", 0)
            if n_warm:
                # dummy matmuls ramp the PE p-state during the pipeline head
                with (
                    tc.tile_pool(name="warm", bufs=1) as warm,
                    tc.tile_pool(name="warmps", bufs=1, space="PSUM") as warmps,
                ):
                    wl = warm.tile([1, 128], f16)
                    wr = warm.tile([1, 512], f16)
                    nc.vector.memset(wl[:, :], 0.0)
                    nc.vector.memset(wr[:, :], 0.0)
                    wps = warmps.tile([128, 512], f32)
                    for _ in range(n_warm):
                        nc.tensor.matmul(wps[:, :], wl[:, :], wr[:, :],
                                         start=True, stop=True)

            wu_sb = consts.tile([128, 4, O], f16)
            nc.sync.dma_start(out=wu_sb[:, :, :], in_=wu_dram[:, :, :])
            wp_sb = consts.tile([128, 2, O], f16)
            nc.sync.dma_start(out=wp_sb[:, :, :], in_=wp_dram[:, :, :])
            bias_sb = consts.tile([O, 1], f32)
            nc.sync.dma_start(out=bias_sb[:, :], in_=b_dram[:, :])

            unit = [0]

            pending_out = []
            cur_out = [None]

            def epilogue(ps, p, j, h, cbase):
                # pack QW-wide epilogue results into an OW-wide out tile;
                # DMA once the tile is full
                off = cbase % OW
                if off == 0 or cur_out[0] is None:
                    cur_out[0] = out_pool.tile([O, OW], f16, name="ot")
                ot = cur_out[0]
                eng = epi[unit[0] % len(epi)]
                unit[0] += 1
                osl = ot[:, off : off + QW]
                # NOTE: GPSIMD cannot access PSUM (BIR verifier) -> act/dve only
                if eng == "act":
                    nc.scalar.activation(
                        osl, ps[:, :], AF.Identity, bias=bias_sb[:, 0:1]
                    )
                else:
                    nc.vector.tensor_scalar_add(osl, ps[:, :], bias_sb[:, 0:1])
                if off + QW == OW:
                    base = F * j + cbase + QW - OW
                    pending_out.append((ot, 2 * p + h, base))
                    cur_out[0] = None

            def flush_out(keep=0):
                # SP queue order: defer out-DMAs so input DMAs never queue
                # behind an out-DMA still waiting on its epilogue.
                while len(pending_out) > keep:
                    ot, row, base = pending_out.pop(0)
                    nc.sync.dma_start(
                        out=y_dram[row, :, base : base + OW], in_=ot[:, :]
                    )

            def process(idx, p, j, c0, w, xin):
                """One sub-tile: columns [c0, c0+w) of partition block p."""
                T1 = t1_pool.tile([128, w], f16, name="T1")
                nc.scalar.activation(T1[:, :], xin[:, :], AF.Tanh)
                T2 = t2_pool.tile([128, w], f16, name="T2")
                nc.vector.tensor_mul(T2[:, :], T1[:, :], T1[:, :])
                T3 = t3_pool.tile([128, w], f16, name="T3")
                nc.vector.tensor_mul(T3[:, :], T1[:, :], T2[:, :])
                T4 = t4_pool.tile([128, w], f16, name="T4")
                if t4_eng[idx] == "act":
                    nc.scalar.activation(T4[:, :], T2[:, :], AF.Square)
                elif t4_eng[idx] == "pool":
                    nc.gpsimd.tensor_mul(T4[:, :], T2[:, :], T2[:, :])
                else:
                    nc.vector.tensor_mul(T4[:, :], T2[:, :], T2[:, :])

                nq = w // QW
                if packed[idx]:
                    A1 = a1_pool.tile([128, w], f16, name="A1")
                    B1 = b1_pool.tile([128, w], f16, name="B1")
                    # order: h=0 operands (clobbered T1, T2) ready first
                    pc = pool_copies[idx]
                    nc.vector.tensor_copy(A1[0:64, :], T1[64:128, :])
                    nc.vector.tensor_copy(T1[64:128, :], T3[0:64, :])
                    nc.vector.tensor_copy(B1[0:64, :], T2[64:128, :])
                    nc.vector.tensor_copy(T2[64:128, :], T4[0:64, :])
                    (nc.gpsimd if pc >= 1 else nc.vector).tensor_copy(
                        A1[64:128, :], T3[64:128, :])
                    (nc.gpsimd if pc >= 2 else nc.vector).tensor_copy(
                        B1[64:128, :], T4[64:128, :])
                    for h in range(2):
                        A, Bt = (T1, T2) if h == 0 else (A1, B1)
                        for q in range(nq):
                            ps = psum_pool.tile([O, QW], f32, name="ps")
                            for c in range(QW // 512):
                                sl = slice(512 * c, 512 * (c + 1))
                                gl = slice(QW * q + 512 * c,
                                           QW * q + 512 * (c + 1))
                                nc.tensor.matmul(
                                    ps[:, sl], wp_sb[:, 0, :], A[:, gl],
                                    start=True, stop=False,
                                )
                                nc.tensor.matmul(
                                    ps[:, sl], wp_sb[:, 1, :], Bt[:, gl],
                                    start=False, stop=True,
                                )
                            epilogue(ps, p, j, h, c0 + QW * q)
                else:
                    pows = (T1, T2, T3, T4)
                    for h in range(2):
                        lo, hi = 64 * h, 64 * (h + 1)
                        pss = [
                            psum_pool.tile([O, QW], f32, name="ps")
                            for q in range(nq)
                        ]
                        # k-outer: k=0 matmuls only need T1 (early PE start)
                        for k in range(4):
                            for c in range(w // 512):
                                ps = pss[c // (QW // 512)]
                                sl = slice(512 * c % QW, 512 * c % QW + 512)
                                gl = slice(512 * c, 512 * (c + 1))
                                nc.tensor.matmul(
                                    ps[:, sl], wu_sb[lo:hi, k, :],
                                    pows[k][lo:hi, gl],
                                    start=(k == 0), stop=(k == 3),
                                )
                        for q in range(nq):
                            epilogue(pss[q], p, j, h, c0 + QW * q)

            for p in range(NP):
                for j in range(NJ):
                    idx = p * NJ + j
                    if idx == 0 and split0:
                        for half in range(2):
                            if half == 0:
                                xin = xin0
                            else:
                                xin = xin_pool.tile([128, F // 2], f16,
                                                    name="xin")
                                nc.sync.dma_start(
                                    out=xin[:, :],
                                    in_=xv[0:128, (F // 2) * half :
                                           (F // 2) * (half + 1)],
                                )
                            process(idx, p, j, (F // 2) * half, F // 2, xin)
                    else:
                        xin = xin_pool.tile([128, F], f16, name="xin")
                        nc.sync.dma_start(
                            out=xin[:, :],
                            in_=xv[128 * p : 128 * (p + 1),
                                   F * j : F * (j + 1)],
                        )
                        flush_out(keep=0)
                        process(idx, p, j, 0, F, xin)
            flush_out()
    nc.compile()
    return nc


_CACHE = {}


def _get_nc(key="default", **kw):
    if key not in _CACHE:
        _CACHE[key] = _build_nc(kw.get("cfg"))
    return _CACHE[key]


def _host_weights(jacobi_coeffs: np.ndarray):
    c = jacobi_coeffs.astype(np.float64)  # (I, O, 5)
    cm = np.einsum("iod,kd->iok", c, _MONO)  # monomial coords, k=0..4
    bias = cm[:, :, 0].sum(axis=0).astype(np.float32).reshape(O, 1)
    w = cm[:, :, 1:].astype(np.float16)  # (I, O, 4) -> k=1..4
    wu = np.empty((128, 4, O), dtype=np.float16)
    wu[0:64] = w.transpose(0, 2, 1)       # [i, k, o]
    wu[64:128] = w.transpose(0, 2, 1)
    wp = np.empty((128, 2, O), dtype=np.float16)
    wp[0:64, 0] = w[:, :, 0]    # W1
    wp[64:128, 0] = w[:, :, 2]  # W3
    wp[0:64, 1] = w[:, :, 1]    # W2
    wp[64:128, 1] = w[:, :, 3]  # W4
    return wu, wp, bias


def kernel(x: np.ndarray, jacobi_coeffs: np.ndarray) -> np.ndarray:
    from concourse.bass_utils import run_bass_kernel_spmd

    wu, wp, bias = _host_weights(np.asarray(jacobi_coeffs))
    x16 = np.asarray(x).astype(np.float16)  # (B, I, S)

    in_maps = []
    for c in range(NCORES):
        xc = np.ascontiguousarray(
            x16[:, :, c * SC : (c + 1) * SC]
        ).reshape(B * I, SC)
        in_maps.append({"x": xc, "wu": wu, "wp": wp, "bias": bias})

    res = run_bass_kernel_spmd(_get_nc(), in_maps, core_ids=list(range(NCORES)))
    y = np.concatenate([r["y"] for r in res.results], axis=2)
    return y.astype(np.float32)



# revision 23
# speedup vs baseline: 1.0009x; 1.0009x over previous
"""KAN (Jacobi/shared) kernel for Trainium2, 8 NeuronCores.

Math: y[b,o,s] = sum_{i,d} P_d(tanh(x[b,i,s])) * C[i,o,d],  P_d = Jacobi(a=1,b=1)
Monomial reformulation (host-side basis change, exact):
  => y[b,o,s] = bias[o] + sum_{k=1..4} sum_i t^k[b,i,s] * W[i,o,k]

Device pipeline (per core, fp16 I/O to halve HBM traffic; tolerance 2e-2):
  DMA x tile [128, F] (2 batches x 64 dims on partitions, F points)
  -> tanh (Act) -> t^2, t^3 (DVE muls), t^4 (Act Square or DVE)
  -> matmuls into 2-bank PSUM units:
     unpacked tiles: 4 accumulating K=64 matmuls per 512-col chunk per batch
     packed tiles:   repack [t1;t3],[t2;t4] per batch via 6 half-copies
                     (DVE 4x f16), then 2 accumulating K=128 matmuls
                     -> halves TensorE time for those tiles
  -> epilogue +bias, f32 PSUM -> f16 SBUF, spread across Act and Pool engines
  -> DMA out.

Sharding: split the 65536-point axis into 8 chunks of 8192 (data parallel),
replicate the tiny weights. Full fp32 inputs in, full fp32 output out
(fp16 conversion on host; device sees fp16).
"""

import sys

import numpy as np

if "/opt/trn_rl_repo" not in sys.path:
    sys.path.insert(0, "/opt/trn_rl_repo")

B = 4
I = 64
S = 65536
O = 128
NCORES = 8
SC = S // NCORES   # 8192 points per core
NP = (B * I) // 128  # 2 partition blocks (2 batches each)
OW = 1024            # out-DMA tile width

# coeff of t^k (rows) in Jacobi P^(1,1)_d (cols), d=0..4
_MONO = np.array(
    [
        [1.0, 0.0, -0.75, 0.0, 0.625],
        [0.0, 2.0, 0.0, -3.0, 0.0],
        [0.0, 0.0, 3.75, 0.0, -8.75],
        [0.0, 0.0, 0.0, 7.0, 0.0],
        [0.0, 0.0, 0.0, 0.0, 13.125],
    ],
    dtype=np.float64,
)

# --- tunables -------------------------------------------------------------
# F: pair-tile width. n_packed: how many pair-tiles (of NP*SC/F) use the
# K=128 packed path. t4_act_n: how many compute t^4 on Act (Square).
# epi = (u_act, u_pool, u_dve): epilogue engine unit counts out of
# 2*B*SC/QW = 32; pool-heavy early, act/dve late.
CONFIG = dict(
    F=2048,
    qw=512,
    n_packed=3,
    packed_mask=(0, 0, 0, 1, 0, 1, 0, 1),
    t4_eng=('pool', 'pool', 'pool', 'act', 'pool', 'dve', 'pool', 'dve'),
    pool_copies=(0, 0, 0, 2, 0, 2, 0, 2),
    epi_list=('act', 'dve', 'act', 'act', 'act', 'dve', 'dve', 'dve', 'act', 'dve', 'dve', 'act', 'dve', 'dve', 'act', 'act', 'act', 'dve', 'act', 'dve', 'act', 'dve', 'act', 'dve', 'dve', 'act', 'act', 'act', 'dve', 'act', 'act', 'act', 'act', 'dve', 'act', 'act', 'act', 'act', 'dve', 'act', 'act', 'act', 'act', 'dve', 'act', 'act', 'act', 'act', 'act', 'act', 'act', 'dve', 'act', 'dve', 'act', 'dve', 'act', 'dve', 'act', 'dve', 'act', 'dve', 'dve', 'act'),
    epi_spec=None,
    split0=True,
    split_last=2,
    n_warm=8,
    bufs={'xin': 4, 't1': 4, 't2': 5, 't3': 3, 't4': 3, 'a1': 2, 'b1': 3, 'out': 6, 'psum': 7},
)
# --------------------------------------------------------------------------


def _spread(n, total):
    """n of `total` slots, spread evenly, avoiding the first and last."""
    picks = set()
    if n >= total:
        return [1] * total
    inner = list(range(1, total))
    step = len(inner) / n if n else 1
    i = 0.0
    while len(picks) < n:
        picks.add(inner[min(int(i), len(inner) - 1)])
        i += step
    return [1 if k in picks else 0 for k in range(total)]


def _epi_order(spec, total):
    """Engine per epilogue unit from segmented spec:
    ((pattern, frac), (pattern, frac), ...) with fracs summing to 1."""
    units = []
    for pat, frac in spec:
        n = round(frac * total)
        for k in range(n):
            units.append(pat[k % len(pat)])
    while len(units) < total:
        units.append("act")
    return units[:total]


def _build_nc(cfg=None):
    import concourse.bacc as bacc
    import concourse.tile as tile
    from concourse import mybir

    cfg = dict(CONFIG if cfg is None else cfg)
    F = cfg["F"]
    NJ = SC // F
    NT = NP * NJ
    bufs = dict(cfg["bufs"])
    split0 = cfg["split0"]
    split_last = cfg.get("split_last", False)
    packed = cfg.get("packed_mask") or _spread(cfg["n_packed"], NT)
    t4_eng = cfg.get("t4_eng") or tuple(
        "act" if b else "dve"
        for b in (cfg.get("t4_mask") or _spread(cfg.get("t4_act_n", 0), NT))
    )
    pool_copies = cfg.get("pool_copies") or (0,) * NT
    QW = cfg.get("qw", 1024)
    n_units = NT * 2 * (F // QW)
    epi = cfg.get("epi_list") or _epi_order(cfg["epi_spec"], n_units)
    f32 = mybir.dt.float32
    f16 = mybir.dt.float16
    AF = mybir.ActivationFunctionType

    nc = bacc.Bacc("TRN2", target_bir_lowering=False, debug=False)

    x_dram = nc.dram_tensor("x", [B * I, SC], f16, kind="ExternalInput")
    wu_dram = nc.dram_tensor("wu", [128, 4, O], f16, kind="ExternalInput")
    wp_dram = nc.dram_tensor("wp", [128, 2, O], f16, kind="ExternalInput")
    b_dram = nc.dram_tensor("bias", [O, 1], f32, kind="ExternalInput")
    y_dram = nc.dram_tensor("y", [B, O, SC], f16, kind="ExternalOutput")

    with tile.TileContext(nc) as tc:
        with (
            tc.tile_pool(name="consts", bufs=1) as consts,
            tc.tile_pool(name="xin", bufs=bufs["xin"]) as xin_pool,
            tc.tile_pool(name="t1", bufs=bufs["t1"]) as t1_pool,
            tc.tile_pool(name="t2", bufs=bufs["t2"]) as t2_pool,
            tc.tile_pool(name="t3", bufs=bufs["t3"]) as t3_pool,
            tc.tile_pool(name="t4", bufs=bufs["t4"]) as t4_pool,
            tc.tile_pool(name="a1", bufs=bufs["a1"]) as a1_pool,
            tc.tile_pool(name="b1", bufs=bufs["b1"]) as b1_pool,
            tc.tile_pool(name="out", bufs=bufs["out"]) as out_pool,
            tc.tile_pool(name="psum", bufs=bufs["psum"], space="PSUM") as psum_pool,
        ):
            xv = x_dram.ap()  # [256, SC]
            # first x chunk before consts: tanh is the pipeline head
            xin0 = xin_pool.tile([128, F // 2], f16, name="xin")
            nc.sync.dma_start(out=xin0[:, :], in_=xv[0:128, 0 : F // 2])

            n_warm = cfg.get("n_warm", 0)
            if n_warm:
                # dummy matmuls ramp the PE p-state during the pipeline head
                with (
                    tc.tile_pool(name="warm", bufs=1) as warm,
                    tc.tile_pool(name="warmps", bufs=1, space="PSUM") as warmps,
                ):
                    wl = warm.tile([1, 128], f16)
                    wr = warm.tile([1, 512], f16)
                    nc.vector.memset(wl[:, :], 0.0)
                    nc.vector.memset(wr[:, :], 0.0)
                    wps = warmps.tile([128, 512], f32)
                    for _ in range(n_warm):
                        nc.tensor.matmul(wps[:, :], wl[:, :], wr[:, :],
                                         start=True, stop=True)

            wu_sb = consts.tile([128, 4, O], f16)
            nc.sync.dma_start(out=wu_sb[:, :, :], in_=wu_dram[:, :, :])
            wp_sb = consts.tile([128, 2, O], f16)
            nc.sync.dma_start(out=wp_sb[:, :, :], in_=wp_dram[:, :, :])
            bias_sb = consts.tile([O, 1], f32)
            nc.sync.dma_start(out=bias_sb[:, :], in_=b_dram[:, :])

            unit = [0]

            pending_out = []
            cur_out = [None]

            def epilogue(ps, p, j, h, cbase):
                # pack QW-wide epilogue results into an OW-wide out tile;
                # DMA once the tile is full
                off = cbase % OW
                if off == 0 or cur_out[0] is None:
                    cur_out[0] = out_pool.tile([O, OW], f16, name="ot")
                ot = cur_out[0]
                eng = epi[unit[0] % len(epi)]
                unit[0] += 1
                osl = ot[:, off : off + QW]
                # NOTE: GPSIMD cannot access PSUM (BIR verifier) -> act/dve only
                if eng == "act":
                    nc.scalar.activation(
                        osl, ps[:, :], AF.Identity, bias=bias_sb[:, 0:1]
                    )
                else:
                    nc.vector.tensor_scalar_add(osl, ps[:, :], bias_sb[:, 0:1])
                if off + QW == OW:
                    base = F * j + cbase + QW - OW
                    pending_out.append((ot, 2 * p + h, base))
                    cur_out[0] = None

            def flush_out(keep=0):
                # SP queue order: defer out-DMAs so input DMAs never queue
                # behind an out-DMA still waiting on its epilogue.
                while len(pending_out) > keep:
                    ot, row, base = pending_out.pop(0)
                    nc.sync.dma_start(
                        out=y_dram[row, :, base : base + OW], in_=ot[:, :]
                    )

            def process(idx, p, j, c0, w, xin):
                """One sub-tile: columns [c0, c0+w) of partition block p."""
                T1 = t1_pool.tile([128, w], f16, name="T1")
                nc.scalar.activation(T1[:, :], xin[:, :], AF.Tanh)
                T2 = t2_pool.tile([128, w], f16, name="T2")
                nc.vector.tensor_mul(T2[:, :], T1[:, :], T1[:, :])
                T3 = t3_pool.tile([128, w], f16, name="T3")
                nc.vector.tensor_mul(T3[:, :], T1[:, :], T2[:, :])
                T4 = t4_pool.tile([128, w], f16, name="T4")
                if t4_eng[idx] == "act":
                    nc.scalar.activation(T4[:, :], T2[:, :], AF.Square)
                elif t4_eng[idx] == "pool":
                    nc.gpsimd.tensor_mul(T4[:, :], T2[:, :], T2[:, :])
                else:
                    nc.vector.tensor_mul(T4[:, :], T2[:, :], T2[:, :])

                nq = w // QW
                if packed[idx]:
                    A1 = a1_pool.tile([128, w], f16, name="A1")
                    B1 = b1_pool.tile([128, w], f16, name="B1")
                    # order: h=0 operands (clobbered T1, T2) ready first
                    pc = pool_copies[idx]
                    nc.vector.tensor_copy(A1[0:64, :], T1[64:128, :])
                    nc.vector.tensor_copy(T1[64:128, :], T3[0:64, :])
                    nc.vector.tensor_copy(B1[0:64, :], T2[64:128, :])
                    nc.vector.tensor_copy(T2[64:128, :], T4[0:64, :])
                    (nc.gpsimd if pc >= 1 else nc.vector).tensor_copy(
                        A1[64:128, :], T3[64:128, :])
                    (nc.gpsimd if pc >= 2 else nc.vector).tensor_copy(
                        B1[64:128, :], T4[64:128, :])
                    for h in range(2):
                        A, Bt = (T1, T2) if h == 0 else (A1, B1)
                        for q in range(nq):
                            ps = psum_pool.tile([O, QW], f32, name="ps")
                            for c in range(QW // 512):
                                sl = slice(512 * c, 512 * (c + 1))
                                gl = slice(QW * q + 512 * c,
                                           QW * q + 512 * (c + 1))
                                nc.tensor.matmul(
                                    ps[:, sl], wp_sb[:, 0, :], A[:, gl],
                                    start=True, stop=False,
                                )
                                nc.tensor.matmul(
                                    ps[:, sl], wp_sb[:, 1, :], Bt[:, gl],
                                    start=False, stop=True,
                                )
                            epilogue(ps, p, j, h, c0 + QW * q)
                else:
                    pows = (T1, T2, T3, T4)
                    for h in range(2):
                        lo, hi = 64 * h, 64 * (h + 1)
                        pss = [
                            psum_pool.tile([O, QW], f32, name="ps")
                            for q in range(nq)
                        ]
                        # k-outer: k=0 matmuls only need T1 (early PE start)
                        for k in range(4):
                            for c in range(w // 512):
                                ps = pss[c // (QW // 512)]
                                sl = slice(512 * c % QW, 512 * c % QW + 512)
                                gl = slice(512 * c, 512 * (c + 1))
                                nc.tensor.matmul(
                                    ps[:, sl], wu_sb[lo:hi, k, :],
                                    pows[k][lo:hi, gl],
                                    start=(k == 0), stop=(k == 3),
                                )
                        for q in range(nq):
                            epilogue(pss[q], p, j, h, c0 + QW * q)

            for p in range(NP):
                for j in range(NJ):
                    idx = p * NJ + j
                    if idx == 0 and split0:
                        for half in range(2):
                            if half == 0:
                                xin = xin0
                            else:
                                xin = xin_pool.tile([128, F // 2], f16,
                                                    name="xin")
                                nc.sync.dma_start(
                                    out=xin[:, :],
                                    in_=xv[0:128, (F // 2) * half :
                                           (F // 2) * (half + 1)],
                                )
                            process(idx, p, j, (F // 2) * half, F // 2, xin)
                    elif idx == NP * NJ - 1 and split_last:
                        # short tail: last tile in pieces so its final
                        # matmuls/epilogue/out-DMA chain is shorter
                        nsp = int(split_last)
                        for piece in range(nsp):
                            pw = F // nsp
                            xin = xin_pool.tile([128, pw], f16, name="xin")
                            nc.sync.dma_start(
                                out=xin[:, :],
                                in_=xv[128 * p : 128 * (p + 1),
                                       F * j + pw * piece :
                                       F * j + pw * (piece + 1)],
                            )
                            flush_out(keep=0)
                            process(idx, p, j, pw * piece, pw, xin)
                    else:
                        xin = xin_pool.tile([128, F], f16, name="xin")
                        nc.sync.dma_start(
                            out=xin[:, :],
                            in_=xv[128 * p : 128 * (p + 1),
                                   F * j : F * (j + 1)],
                        )
                        flush_out(keep=0)
                        process(idx, p, j, 0, F, xin)
            flush_out()
    nc.compile()
    return nc


_CACHE = {}


def _get_nc(key="default", **kw):
    if key not in _CACHE:
        _CACHE[key] = _build_nc(kw.get("cfg"))
    return _CACHE[key]


def _host_weights(jacobi_coeffs: np.ndarray):
    c = jacobi_coeffs.astype(np.float64)  # (I, O, 5)
    cm = np.einsum("iod,kd->iok", c, _MONO)  # monomial coords, k=0..4
    bias = cm[:, :, 0].sum(axis=0).astype(np.float32).reshape(O, 1)
    w = cm[:, :, 1:].astype(np.float16)  # (I, O, 4) -> k=1..4
    wu = np.empty((128, 4, O), dtype=np.float16)
    wu[0:64] = w.transpose(0, 2, 1)       # [i, k, o]
    wu[64:128] = w.transpose(0, 2, 1)
    wp = np.empty((128, 2, O), dtype=np.float16)
    wp[0:64, 0] = w[:, :, 0]    # W1
    wp[64:128, 0] = w[:, :, 2]  # W3
    wp[0:64, 1] = w[:, :, 1]    # W2
    wp[64:128, 1] = w[:, :, 3]  # W4
    return wu, wp, bias


def kernel(x: np.ndarray, jacobi_coeffs: np.ndarray) -> np.ndarray:
    from concourse.bass_utils import run_bass_kernel_spmd

    wu, wp, bias = _host_weights(np.asarray(jacobi_coeffs))
    x16 = np.asarray(x).astype(np.float16)  # (B, I, S)

    in_maps = []
    for c in range(NCORES):
        xc = np.ascontiguousarray(
            x16[:, :, c * SC : (c + 1) * SC]
        ).reshape(B * I, SC)
        in_maps.append({"x": xc, "wu": wu, "wp": wp, "bias": bias})

    res = run_bass_kernel_spmd(_get_nc(), in_maps, core_ids=list(range(NCORES)))
    y = np.concatenate([r["y"] for r in res.results], axis=2)
    return y.astype(np.float32)

